# revision 9
# baseline (speedup 1.0000x reference)
"""Trainium2 Bass kernel for multi-head attention (B=16, C=512, H=W=32, 8 heads).

Sharding: pure data-parallel over batch — each of the 8 NeuronCores gets 2
batches; weights are replicated. No collectives.

Per-core algorithm (per batch b):
  x[b] arrives as (C=512, S=1024) — already the transposed activation layout
  the TensorEngine wants (contraction dim on partitions).

  1. qkT = Wqk @ x[b]            -> (1024, S)   q rows 0..511, k rows 512..1023
  2. v   = x[b].T @ WvT          -> (S, 512)    (s on partitions)
     v_ext[s, h, 0:64] = v head h, v_ext[s, h, 64] = 1.0   (ones column)
  3. per head h (hd=64):
       logitsT[kpos, q] = kT_h.T-contracted matmul (K=hd)  (k on partitions!)
       explT = exp(0.125 * logitsT)            (ScalarE, no max subtraction --
                                                logits ~ N(0,1), max < ~6)
       po = v_ext_h.T @ explT                  -> (65, S): rows 0..63 = o^T_h
                                                  row 64 = sum_k explT  (fused)
       oT_h = po[0:64] * broadcast(1/po[64])   (recip bcast via K=1 matmul)
  4. outT = WoutT.T @ o^T  (contract over c_in, K=64 per head, accumulated)
     outT is (C, S) == the NCHW output layout. DMA out.

Compute in bf16 (f32 PSUM accumulation); harness tolerance is ~2e-2.
"""

import numpy as np
import ml_dtypes

import concourse.bass as bass
from concourse import bacc
import concourse.mybir as mybir
from concourse.tile import TileContext
from concourse.bass_utils import run_bass_kernel_spmd

F32 = mybir.dt.float32
BF16 = mybir.dt.bfloat16

B, C, S = 16, 512, 1024
NH, HD = 8, 64
NCORES = 8
BPC = B // NCORES  # batches per core
KT = C // 128      # 4   k-tiles of the c_in contraction
MT_QK = 2 * C // 128  # 8 row-tiles of the qk projection output
ST = S // 128      # 8   s-tiles
NT = S // 512      # 2   512-wide moving chunks

LAST_EXEC_TIME_NS = None
_NC_CACHE = {}


def _build_nc():
    nc = bacc.Bacc(trn_type="TRN2", target_bir_lowering=False)

    x_ext = nc.declare_dram_parameter("x", [BPC, C, S], F32, isOutput=False)
    wqk_ext = nc.declare_dram_parameter("wqk_t", [C, 2 * C], BF16, isOutput=False)
    wv_ext = nc.declare_dram_parameter("wv_t", [C, C], BF16, isOutput=False)
    wo_ext = nc.declare_dram_parameter("wout_t", [C, C], BF16, isOutput=False)
    out_ext = nc.declare_dram_parameter("out", [BPC, C, S], F32, isOutput=True)

    with TileContext(nc) as tc:
        with (
            tc.tile_pool(name="const", bufs=1) as const,
            tc.tile_pool(name="acts", bufs=2) as acts,
            tc.tile_pool(name="expl", bufs=2) as expl_pool,
            tc.tile_pool(name="oT", bufs=10) as oT_pool,
            tc.tile_pool(name="rc", bufs=2) as rc_pool,
            tc.tile_pool(name="osb", bufs=8) as osb_pool,
            tc.tile_pool(name="psl", bufs=3, space="PSUM") as psl,
            tc.tile_pool(name="pso", bufs=2, space="PSUM") as pso,
            tc.tile_pool(name="psr", bufs=1, space="PSUM") as psr,
        ):
            # ---- weights (bf16 straight from HBM) ----
            wqk_bf = const.tile([128, KT, 2 * C], BF16, name="wqk_bf")
            nc.sync.dma_start(
                out=wqk_bf, in_=wqk_ext[:, :].rearrange("(kt p) n -> p kt n", p=128)
            )
            wv_bf = const.tile([128, KT, C], BF16, name="wv_bf")
            nc.sync.dma_start(
                out=wv_bf, in_=wv_ext[:, :].rearrange("(kt p) n -> p kt n", p=128)
            )
            wo_bf = const.tile([HD, NH, C], BF16, name="wo_bf")
            nc.sync.dma_start(
                out=wo_bf, in_=wo_ext[:, :].rearrange("(h p) n -> p h n", p=HD)
            )
            ones_bf = const.tile([1, HD], BF16, name="ones_bf")
            nc.vector.memset(ones_bf, 1.0)

            for b in range(BPC):
                # ---- load x[b], casting f32->bf16 in the DMA (SWDGE casts) ----
                xb = acts.tile([128, KT, S], BF16, tag="xb", name="xb")
                x_dram = x_ext[b, :, :].rearrange("(ct p) s -> p ct s", p=128)
                for ct in range(KT):
                    nc.gpsimd.dma_start(out=xb[:, ct, :], in_=x_dram[:, ct, :])

                # ---- q/k projection: qkT[c, s], c in [0, 1024) ----
                qkT = acts.tile([128, MT_QK, S], BF16, tag="qkT", name="qkT")
                for mt in range(MT_QK):
                    for nt in range(NT):
                        ps = psl.tile([128, 512], F32, tag="ps", name="ps_qk")
                        for kt in range(KT):
                            nc.tensor.matmul(
                                ps,
                                lhsT=wqk_bf[:, kt, mt * 128:(mt + 1) * 128],
                                rhs=xb[:, kt, nt * 512:(nt + 1) * 512],
                                start=(kt == 0),
                                stop=(kt == KT - 1),
                            )
                        nc.scalar.copy(
                            out=qkT[:, mt, nt * 512:(nt + 1) * 512], in_=ps
                        )

                # ---- v projection (s on partitions) + ones column ----
                v_ext = acts.tile([128, ST, NH, HD + 1], BF16, tag="vext", name="v_ext")
                nc.vector.memset(v_ext[:, :, :, HD:HD + 1], 1.0)
                for st in range(ST):
                    ps = psl.tile([128, 512], F32, tag="ps", name="ps_v")
                    for kt in range(KT):
                        nc.tensor.matmul(
                            ps,
                            lhsT=xb[:, kt, st * 128:(st + 1) * 128],
                            rhs=wv_bf[:, kt, :],
                            start=(kt == 0),
                            stop=(kt == KT - 1),
                        )
                    nc.vector.tensor_copy(
                        out=v_ext[:, st, :, 0:HD],
                        in_=ps.rearrange("p (h d) -> p h d", h=NH),
                    )

                # ---- attention per head ----
                oT_tiles = []
                for h in range(NH):
                    po_base = (h % 2) * 64
                    qh = qkT[po_base:po_base + 64, h // 2, :]
                    kh = qkT[po_base:po_base + 64, NH // 2 + h // 2, :]

                    ex = expl_pool.tile([128, ST, S], BF16, tag="ex", name="ex")
                    for kt in range(ST):
                        for nt in range(NT):
                            ps = psl.tile([128, 512], F32, tag="ps", name="ps_l")
                            nc.tensor.matmul(
                                ps,
                                lhsT=kh[:, kt * 128:(kt + 1) * 128],
                                rhs=qh[:, nt * 512:(nt + 1) * 512],
                                start=True,
                                stop=True,
                            )
                            nc.scalar.activation(
                                out=ex[:, kt, nt * 512:(nt + 1) * 512],
                                in_=ps,
                                func=mybir.ActivationFunctionType.Exp,
                                scale=0.125,
                            )

                    po = pso.tile([HD + 1, S], F32, tag="po", name="po")
                    for nt in range(NT):
                        for kt in range(ST):
                            nc.tensor.matmul(
                                po[:, nt * 512:(nt + 1) * 512],
                                lhsT=v_ext[:, kt, h, :],
                                rhs=ex[:, kt, nt * 512:(nt + 1) * 512],
                                start=(kt == 0),
                                stop=(kt == ST - 1),
                            )

                    rcf = rc_pool.tile([1, S], F32, tag="rcf", name="rcf")
                    nc.vector.reciprocal(rcf, po[HD:HD + 1, :])
                    rcb = rc_pool.tile([1, S], BF16, tag="rcb", name="rcb")
                    nc.vector.tensor_copy(out=rcb, in_=rcf)

                    oT = oT_pool.tile([HD, S], BF16, tag="oT", name="oT")
                    for nt in range(NT):
                        pr = psr.tile([HD, 512], F32, tag="pr", name="pr")
                        nc.tensor.matmul(
                            pr,
                            lhsT=ones_bf,
                            rhs=rcb[0:1, nt * 512:(nt + 1) * 512],
                            start=True,
                            stop=True,
                        )
                        rb = rc_pool.tile([HD, 512], BF16, tag="rb", name="rb")
                        nc.vector.tensor_copy(out=rb, in_=pr)
                        nc.vector.tensor_mul(
                            oT[:, nt * 512:(nt + 1) * 512],
                            po[0:HD, nt * 512:(nt + 1) * 512],
                            rb,
                        )
                    oT_tiles.append(oT)

                # ---- output projection: outT[c_out, s] (contract c_in = heads) ----
                out_dram = out_ext[b, :, :].rearrange("(mt p) s -> p mt s", p=128)
                for mt in range(KT):
                    out_sb = osb_pool.tile([128, S], F32, tag="osb", name="out_sb")
                    for nt in range(NT):
                        ps = psl.tile([128, 512], F32, tag="ps", name="ps_o")
                        for h in range(NH):
                            nc.tensor.matmul(
                                ps,
                                lhsT=wo_bf[:, h, mt * 128:(mt + 1) * 128],
                                rhs=oT_tiles[h][:, nt * 512:(nt + 1) * 512],
                                start=(h == 0),
                                stop=(h == NH - 1),
                            )
                        nc.scalar.copy(
                            out=out_sb[:, nt * 512:(nt + 1) * 512], in_=ps
                        )
                    nc.sync.dma_start(out=out_dram[:, mt, :], in_=out_sb)

    nc.compile()
    return nc


def _get_nc():
    if "nc" not in _NC_CACHE:
        _NC_CACHE["nc"] = _build_nc()
    return _NC_CACHE["nc"]


def kernel(x, w_qkv, w_out):
    global LAST_EXEC_TIME_NS
    x = np.ascontiguousarray(np.asarray(x, dtype=np.float32)).reshape(B, C, S)
    w_qkv = np.asarray(w_qkv, dtype=np.float32)
    w_out = np.asarray(w_out, dtype=np.float32)

    wqk_t = np.ascontiguousarray(w_qkv[: 2 * C].T).astype(ml_dtypes.bfloat16)
    wv_t = np.ascontiguousarray(w_qkv[2 * C:].T).astype(ml_dtypes.bfloat16)
    wout_t = np.ascontiguousarray(w_out.T).astype(ml_dtypes.bfloat16)

    nc = _get_nc()
    in_maps = [
        {
            "x": x[i * BPC:(i + 1) * BPC],
            "wqk_t": wqk_t,
            "wv_t": wv_t,
            "wout_t": wout_t,
        }
        for i in range(NCORES)
    ]
    res = run_bass_kernel_spmd(nc, in_maps, core_ids=list(range(NCORES)))
    LAST_EXEC_TIME_NS = res.exec_time_ns
    out = np.concatenate([res.results[i]["out"] for i in range(NCORES)], axis=0)
    return out.reshape(B, C, 32, 32)


if __name__ == "__main__":
    _build_nc()
    print("build OK")


# revision 27
# speedup vs baseline: 8.9424x; 8.9424x over previous
"""Trainium2 Bass kernel for multi-head attention (B=16, C=512, H=W=32, 8 heads).

Sharding: pure data-parallel over batch — each of the 8 NeuronCores gets 2
batches; weights are replicated. No collectives.

Per-core algorithm (per batch b):
  x[b] arrives as (C=512, S=1024) — already the transposed activation layout
  the TensorEngine wants (contraction dim on partitions).

  1. qkT = Wqk @ x[b]            -> (1024, S)   q rows 0..511, k rows 512..1023
  2. v   = x[b].T @ WvT          -> (S, 512)    (s on partitions)
     v_ext[s, h, 0:64] = v head h, v_ext[s, h, 64] = 1.0   (ones column)
  3. per head h (hd=64), heads processed in pairs at partition bases 0/64 so
     their K=64 QK matmuls land in distinct PE row-groups and run concurrently:
       logitsT[kpos, q] (k on partitions, PSUM)
       explT = exp(0.125 * logitsT)            (ScalarE, no max subtraction --
                                                logits ~ N(0,1), max < ~6)
       po = v_ext_h.T @ explT                  -> (65, S): rows 0..63 = o^T_h
                                                  row 64 = sum_k explT  (fused)
       oT_h = po[0:64] * bcast(1/po[64])       (recip on DVE, partition
                                                broadcast on GpSimd)
  4. outT = WoutT.T @ o^T  (contract over c_in, K=64 per head, accumulated)
     outT is (C, S) == the NCHW output layout. DMA out.

Compute in bf16 (f32 PSUM accumulation); harness tolerance is ~2e-2.
"""

import numpy as np
import ml_dtypes

import concourse.bass as bass
from concourse import bacc
import concourse.mybir as mybir
from concourse.tile import TileContext
from concourse.bass_utils import run_bass_kernel_spmd

F32 = mybir.dt.float32
BF16 = mybir.dt.bfloat16

B, C, S = 16, 512, 1024
NH, HD = 8, 64
NCORES = 8
BPC = B // NCORES  # batches per core
KT = C // 128      # 4   k-tiles of the c_in contraction
MT_QK = 2 * C // 128  # 8 row-tiles of the qk projection output
ST = S // 128      # 8   s-tiles
NT = S // 512      # 2   512-wide chunks

LAST_EXEC_TIME_NS = None
_NC_CACHE = {}


def _build_nc(reps=1, skip_proj=False, skip_attn=False):
    nc = bacc.Bacc(trn_type="TRN2", target_bir_lowering=False)

    x_ext = nc.declare_dram_parameter("x", [BPC, C, S], F32, isOutput=False)
    wqk_ext = nc.declare_dram_parameter("wqk_t", [C, 2 * C], BF16, isOutput=False)
    wv_ext = nc.declare_dram_parameter("wv_t", [C, C], BF16, isOutput=False)
    wo_ext = nc.declare_dram_parameter("wout_t", [C, C], BF16, isOutput=False)
    out_ext = nc.declare_dram_parameter("out", [BPC, C, S], F32, isOutput=True)

    with TileContext(nc) as tc:
        with (
            tc.tile_pool(name="const", bufs=1) as const,
            tc.tile_pool(name="acts", bufs=2) as acts,
            tc.tile_pool(name="expl", bufs=4) as expl_pool,
            tc.tile_pool(name="oT", bufs=6) as oT_pool,
            tc.tile_pool(name="rc", bufs=2) as rc_pool,
            tc.tile_pool(name="osb", bufs=2) as osb_pool,
            tc.tile_pool(name="psl", bufs=6, space="PSUM") as psl,
            tc.tile_pool(name="pso", bufs=2, space="PSUM") as pso,
        ):
            # ---- weights (bf16 straight from HBM) ----
            wqk_bf = const.tile([128, KT, 2 * C], BF16, name="wqk_bf")
            nc.sync.dma_start(
                out=wqk_bf, in_=wqk_ext[:, :].rearrange("(kt p) n -> p kt n", p=128)
            )
            wv_bf = const.tile([128, KT, C], BF16, name="wv_bf")
            nc.sync.dma_start(
                out=wv_bf, in_=wv_ext[:, :].rearrange("(kt p) n -> p kt n", p=128)
            )
            wo_bf = const.tile([128, KT, C], BF16, name="wo_bf")
            nc.sync.dma_start(
                out=wo_bf, in_=wo_ext[:, :].rearrange("(kt p) n -> p kt n", p=128)
            )
            # v_ext[s, st, h, 0:64] = v head h; cols 64:128 stay 1.0 so the AV
            # matmul replicates the softmax denominator into rows 64:128.
            v_ext_tiles = []
            for i in range(2):
                v_ext = const.tile([128, ST, NH, 128], BF16, name=f"v_ext{i}")
                nc.vector.memset(v_ext[:, :, :, HD:], 1.0)
                if skip_proj:
                    nc.vector.memset(v_ext[:, :, :, 0:HD], 0.01)
                v_ext_tiles.append(v_ext)

            # ---- software pipeline: next batch's load+projections interleave
            # with this batch's attention pairs so ACT never drains ----
            seq = [i % BPC for i in range(reps * BPC)]
            state = {}

            def emit_load(i):
                b = seq[i]
                xb = acts.tile([128, KT, S], BF16, tag="xb", name="xb")
                x_dram = x_ext[b, :, :].rearrange("(ct p) s -> p ct s", p=128)
                for ct in range(KT):
                    x_f = acts.tile([128, S], F32, tag="xf", name="x_f", bufs=2)
                    nc.sync.dma_start(out=x_f, in_=x_dram[:, ct, :])
                    nc.vector.tensor_copy(out=xb[:, ct, :], in_=x_f)
                qkT = acts.tile([128, MT_QK, S], BF16, tag="qkT", name="qkT")
                if skip_proj:
                    nc.vector.memset(qkT, 0.02)
                state[i] = {"xb": xb, "qkT": qkT, "v_ext": v_ext_tiles[i % 2]}

            def emit_proj_chunk(i, q):
                if skip_proj:
                    return
                s = state[i]
                xb, qkT, v_ext = s["xb"], s["qkT"], s["v_ext"]
                if q < 2:
                    for mt in range(4 * q, 4 * q + 4):
                        for nt in range(NT):
                            nsl = slice(nt * 512, (nt + 1) * 512)
                            ps = psl.tile([128, 512], F32, tag="ps", name="ps_qk")
                            for kt in range(KT):
                                nc.tensor.matmul(
                                    ps,
                                    lhsT=wqk_bf[:, kt, mt * 128:(mt + 1) * 128],
                                    rhs=xb[:, kt, nsl],
                                    start=(kt == 0),
                                    stop=(kt == KT - 1),
                                )
                            nc.vector.tensor_copy(out=qkT[:, mt, nsl], in_=ps)
                else:
                    for st in range(4 * (q - 2), 4 * (q - 2) + 4):
                        ps = psl.tile([128, C], F32, tag="ps", name="ps_v")
                        for kt in range(KT):
                            nc.tensor.matmul(
                                ps,
                                lhsT=xb[:, kt, st * 128:(st + 1) * 128],
                                rhs=wv_bf[:, kt, :],
                                start=(kt == 0),
                                stop=(kt == KT - 1),
                            )
                        nc.vector.tensor_copy(
                            out=v_ext[:, st, :, 0:HD],
                            in_=ps.rearrange("p (h d) -> p h d", h=NH),
                        )

            def emit_pair(i, hp):
                if skip_attn:
                    oT2 = oT_pool.tile([128, S], BF16, tag="oT", name="oT2")
                    nc.vector.tensor_copy(out=oT2, in_=state[i]["qkT"][:, hp, :])
                    state[i].setdefault("oT", []).append(oT2)
                    return
                s = state[i]
                qkT, v_ext = s["qkT"], s["v_ext"]
                hA, hB = 2 * hp, 2 * hp + 1
                qA = qkT[0:64, hp, :]
                kA = qkT[0:64, NH // 2 + hp, :]
                qB = qkT[64:128, hp, :]
                kB = qkT[64:128, NH // 2 + hp, :]

                exA = expl_pool.tile([128, ST, S], BF16, tag="ex", name="exA")
                exB = expl_pool.tile([128, ST, S], BF16, tag="ex", name="exB")
                for kt in range(ST):
                    ksl = slice(kt * 128, (kt + 1) * 128)
                    for nt in range(NT):
                        nsl = slice(nt * 512, (nt + 1) * 512)
                        psA = psl.tile([128, 512], F32, tag="ps", name="ps_lA")
                        psB = psl.tile([128, 512], F32, tag="ps", name="ps_lB")
                        nc.tensor.matmul(psA, lhsT=kA[:, ksl],
                                         rhs=qA[:, nsl], start=True, stop=True)
                        nc.tensor.matmul(psB, lhsT=kB[:, ksl],
                                         rhs=qB[:, nsl], start=True, stop=True)
                        nc.scalar.activation(
                            out=exA[:, kt, nsl], in_=psA,
                            func=mybir.ActivationFunctionType.Exp, scale=0.125)
                        nc.scalar.activation(
                            out=exB[:, kt, nsl], in_=psB,
                            func=mybir.ActivationFunctionType.Exp, scale=0.125)

                oT2 = oT_pool.tile([128, S], BF16, tag="oT", name="oT2")
                for idx, (h, ex) in enumerate(((hA, exA), (hB, exB))):
                    for nt in range(NT):
                        nsl = slice(nt * 512, (nt + 1) * 512)
                        po = pso.tile([128, 512], F32, tag="po", name="po")
                        for kt in range(ST):
                            nc.tensor.matmul(
                                po,
                                lhsT=v_ext[:, kt, h, :],
                                rhs=ex[:, kt, nsl],
                                start=(kt == 0),
                                stop=(kt == ST - 1),
                            )
                        rb = rc_pool.tile([HD, 512], BF16, tag="rb",
                                          name="rb", bufs=2)
                        with nc.allow_low_precision(reason="bf16 denom"):
                            nc.vector.reciprocal(rb, po[HD:, :])
                        nc.vector.tensor_mul(
                            oT2[idx * HD:(idx + 1) * HD, nsl], po[0:HD, :], rb
                        )
                s.setdefault("oT", []).append(oT2)

            def emit_outproj(i):
                b = seq[i]
                oT_tiles = state[i]["oT"]
                out_dram = out_ext[b, :, :].rearrange("(mt p) s -> p mt s", p=128)
                for mt in range(KT):
                    out_sb = osb_pool.tile([128, S], F32, tag="osb", name="out_sb")
                    for nt in range(NT):
                        nsl = slice(nt * 512, (nt + 1) * 512)
                        ps = psl.tile([128, 512], F32, tag="ps", name="ps_o")
                        for j in range(KT):
                            nc.tensor.matmul(
                                ps,
                                lhsT=wo_bf[:, j, mt * 128:(mt + 1) * 128],
                                rhs=oT_tiles[j][:, nsl],
                                start=(j == 0),
                                stop=(j == KT - 1),
                            )
                        nc.vector.tensor_copy(out=out_sb[:, nsl], in_=ps)
                    nc.sync.dma_start(out=out_dram[:, mt, :], in_=out_sb)
                del state[i]

            emit_load(0)
            for q in range(4):
                emit_proj_chunk(0, q)
            for i in range(len(seq)):
                if i + 1 < len(seq):
                    emit_load(i + 1)
                for hp in range(NH // 2):
                    emit_pair(i, hp)
                    if i + 1 < len(seq):
                        emit_proj_chunk(i + 1, hp)
                emit_outproj(i)

    nc.compile()
    return nc


def _get_nc(reps=1):
    if reps not in _NC_CACHE:
        _NC_CACHE[reps] = _build_nc(reps)
    return _NC_CACHE[reps]


def kernel(x, w_qkv, w_out):
    global LAST_EXEC_TIME_NS
    x = np.ascontiguousarray(np.asarray(x, dtype=np.float32)).reshape(B, C, S)
    w_qkv = np.asarray(w_qkv, dtype=np.float32)
    w_out = np.asarray(w_out, dtype=np.float32)

    wqk_t = np.ascontiguousarray(w_qkv[: 2 * C].T).astype(ml_dtypes.bfloat16)
    wv_t = np.ascontiguousarray(w_qkv[2 * C:].T).astype(ml_dtypes.bfloat16)
    wout_t = np.ascontiguousarray(w_out.T).astype(ml_dtypes.bfloat16)

    nc = _get_nc()
    in_maps = [
        {
            "x": x[i * BPC:(i + 1) * BPC],
            "wqk_t": wqk_t,
            "wv_t": wv_t,
            "wout_t": wout_t,
        }
        for i in range(NCORES)
    ]
    res = run_bass_kernel_spmd(nc, in_maps, core_ids=list(range(NCORES)))
    LAST_EXEC_TIME_NS = res.exec_time_ns
    out = np.concatenate([res.results[i]["out"] for i in range(NCORES)], axis=0)
    return out.reshape(B, C, 32, 32)


if __name__ == "__main__":
    _build_nc()
    print("build OK")
